# revision 9
# baseline (speedup 1.0000x reference)
"""Trainium2 Bass kernel for nn_DetectionHead (class-margin NMS).

reference:
    x  = seg[:,1] - seg[:,0] - EPS
    xp = relu(x)
    xm = 3x3 hole max-pool of xp (8 neighbors, zero-padded), max'ed with 0
    out = x * (x > xm)                       ;  returns (out, seg)

Identity used: max(0, pool(relu(x))) == max(0, pool_{0-pad}(x)), so the pool
runs on raw x with zero padding and a single max(., 0) clamp — every value on
the comparison path stays bit-exact with the fp32 reference.

Sharding: pure data-parallel over batch (16 images -> 8 cores x 2). Per-core
layout: H on partitions in chunks of 126 output rows (+1 halo row each side),
both images side by side along the free dim (W columns each). Horizontal
neighbor maxes are shifted free-dim APs on a zero-padded x tile; vertical +-1
row shifts cross partitions, which compute engines cannot do, so they run on
the PE as fp32 matmuls with shifted-identity weights (permutation weights are
bit-exact through the 4-pass fp32 path).

Custom fused DVE ops (registered at import):
    MARGIN_SUB_NMS_ANT: x = (c1 - c0) - eps          (one op, ref rounding)
    MASK_MUL_NMS_ANT:   out = select(x > xm, x, 0)   (mask+mul in one op)
"""

import numpy as np

import concourse.bacc as bacc
import concourse.dve_ops as dve_ops
import concourse.mybir as mybir
from concourse.dve_ops import DveOp, get_dve_sub_opcode, has_src1
from concourse.dve_spec import Spec, Src0, Src1, C0, Zero, lower, select
from concourse.dve_uop import DveOpSpec
from concourse.tile import TileContext
from concourse.bass_utils import run_bass_kernel_spmd

EPS = 0.01
B, C, H, W = 16, 2, 1024, 1024
N_CORES = 8
B_PER = B // N_CORES  # images per core
CHUNK = 126  # output rows per full chunk (128 partitions - 2 halo rows)
WP = W + 2  # padded width

F32 = mybir.dt.float32
ALU = mybir.AluOpType


def _register_dve_op(name: str, spec: Spec) -> DveOp:
    for op in dve_ops.OPS:
        if op.name == name:
            return op
    op = DveOp(name, spec, subdim=False, uops_sha={})
    dve_ops.OPS.append(op)
    dve_ops._SUB_OPCODE_FOR_NAME[name] = dve_ops._CUSTOM_DVE_ROW_BASE + len(dve_ops.OPS) - 1
    dve_ops.CUSTOM_DVE_SPECS[name] = spec
    for ver in ("v3", "v4"):
        compiled = DveOpSpec(
            name=name,
            opcode=get_dve_sub_opcode(name),
            uops=lower(spec, ver=ver),
            rd1_en=has_src1(spec),
        )
        op.uops_sha[ver] = compiled.sha(ver)
    return op


MARGIN_SUB_NMS_ANT = _register_dve_op(
    "MARGIN_SUB_NMS_ANT",
    Spec(
        body=(Src0 - Src1) - C0,
        reference=lambda in0, in1, s0, s1, imm2: (in0 - in1) - s0,
    ),
)

MASK_MUL_NMS_ANT = _register_dve_op(
    "MASK_MUL_NMS_ANT",
    Spec(
        body=select(Src0 > Src1, Src0, Zero),
        reference=lambda in0, in1, s0, s1, imm2: np.where(in0 > in1, in0, np.float32(0.0)),
    ),
)


def _image_chunks():
    """Per-image chunk list: (r0, r1, or0, or1, po).

    rows [r0, r1) are loaded to partitions [0, r1-r0); output rows
    [or0, or1) live at partitions [po, po + (or1-or0)).
    """
    chunks = [(0, 127, 0, 126, 0)]
    for c in range(1, (H + CHUNK - 1) // CHUNK):
        s = CHUNK * c
        chunks.append((s - 1, min(s + CHUNK + 1, H), s, min(s + CHUNK, H), 1))
    return chunks


def _build_nc():
    nc = bacc.Bacc("TRN2", target_bir_lowering=False, debug=False)
    seg = nc.dram_tensor("segmentation", [B_PER, C, H, W], F32, kind="ExternalInput")
    su = nc.dram_tensor("shift_up", [128, 128], F32, kind="ExternalInput")
    sd = nc.dram_tensor("shift_dn", [128, 128], F32, kind="ExternalInput")
    out = nc.dram_tensor("xnms", [B_PER, H, W], F32, kind="ExternalOutput")

    seg_ap = seg.ap()
    out_ap = out.ap()
    FW = B_PER * W  # free width of working tiles (both images side by side)

    with TileContext(nc) as tc:
        with (
            tc.tile_pool(name="wpool", bufs=1) as wpool,
            tc.tile_pool(name="inp", bufs=3) as inp,
            tc.tile_pool(name="xsp", bufs=2) as xsp,
            tc.tile_pool(name="tmp", bufs=2) as tmp,
            tc.tile_pool(name="psu", bufs=1, space="PSUM") as psu,
            tc.tile_pool(name="psd", bufs=1, space="PSUM") as psd,
            tc.tile_pool(name="outp", bufs=3) as outp,
        ):
            su_t = wpool.tile([128, 128], F32, tag="su")
            sd_t = wpool.tile([128, 128], F32, tag="sd")
            nc.sync.dma_start(out=su_t[:], in_=su.ap())
            nc.sync.dma_start(out=sd_t[:], in_=sd.ap())

            for (r0, r1, or0, or1, po) in _image_chunks():
                nr = r1 - r0
                orows = or1 - or0

                # one DMA: both images, both channels -> [128, B_PER*C*W]
                it = inp.tile([128, B_PER * C * W], F32, tag="it")
                nc.sync.dma_start(
                    out=it[:nr, :].rearrange("p (q w) -> p q w", q=B_PER * C),
                    in_=seg_ap.rearrange("b c h w -> h (b c) w")[r0:r1, :, :],
                )
                it4 = it[:nr, :].rearrange("p (b c w) -> p b c w", b=B_PER, c=C)

                # x = (c1 - c0) - EPS, rounded exactly like the reference
                xt = xsp.tile([128, FW], F32, tag="xt")
                xc = xt[:nr, :].rearrange("p (b w) -> p b w", b=B_PER)
                nc.vector._custom_dve(
                    MARGIN_SUB_NMS_ANT,
                    out=xc,
                    in0=it4[:, :, 1, :],
                    in1=it4[:, :, 0, :],
                    s0=EPS,
                )

                # t1 = max(x[j-1], x[j+1])  (0-padded at the image edges; the
                # pad zero is absorbed by the global max(.,0) clamp)
                t1 = tmp.tile([128, FW], F32, tag="t1")
                t13 = t1[:nr, :].rearrange("p (b w) -> p b w", b=B_PER)
                nc.vector.tensor_max(
                    t13[:, :, 1 : W - 1], xc[:, :, 0 : W - 2], xc[:, :, 2:W]
                )
                nc.vector.tensor_scalar_max(t13[:, :, 0:1], xc[:, :, 1:2], 0.0)
                nc.vector.tensor_scalar_max(
                    t13[:, :, W - 1 : W], xc[:, :, W - 2 : W - 1], 0.0
                )
                # m3 = max(t1, x[j])
                m3 = tmp.tile([128, FW], F32, tag="m3")
                nc.vector.tensor_max(m3[:nr], t1[:nr], xt[:nr])

                # vertical +-1 shifts of m3 through the PE (exact fp32)
                up = psu.tile([128, FW], F32, tag="up")
                dn = psd.tile([128, FW], F32, tag="dn")
                for h0 in range(0, FW, 512):
                    nc.tensor.matmul(
                        up[:, h0 : h0 + 512],
                        su_t[:nr, :],
                        m3[:nr, h0 : h0 + 512],
                        start=True,
                        stop=True,
                    )
                    nc.tensor.matmul(
                        dn[:, h0 : h0 + 512],
                        sd_t[:nr, :],
                        m3[:nr, h0 : h0 + 512],
                        start=True,
                        stop=True,
                    )

                # xm = max(up, dn, t1, 0) — one PSUM operand per instruction
                a = tmp.tile([128, FW], F32, tag="a")
                nc.vector.scalar_tensor_tensor(
                    a[:nr], up[:nr], 0.0, t1[:nr], ALU.max, ALU.max
                )
                xm = tmp.tile([128, FW], F32, tag="xm")
                nc.vector.tensor_max(xm[:nr], a[:nr], dn[:nr])

                # out = x * (x > xm)
                ot = outp.tile([128, FW], F32, tag="ot")
                nc.vector._custom_dve(
                    MASK_MUL_NMS_ANT,
                    out=ot[:nr, :],
                    in0=xt[:nr, :],
                    in1=xm[:nr, :],
                )

                nc.sync.dma_start(
                    out=out_ap.rearrange("b h w -> h b w")[or0:or1, :, :],
                    in_=ot[po : po + orows, :].rearrange(
                        "p (b w) -> p b w", b=B_PER
                    ),
                )

    nc.compile()
    return nc


_NC = None


def _get_nc():
    global _NC
    if _NC is None:
        _NC = _build_nc()
    return _NC


def kernel(segmentation: np.ndarray) -> tuple[np.ndarray, np.ndarray]:
    seg = np.ascontiguousarray(segmentation, dtype=np.float32)
    su = np.eye(128, k=-1, dtype=np.float32)  # out[p] = in[p+1]
    sd = np.eye(128, k=1, dtype=np.float32)  # out[p] = in[p-1]
    nc = _get_nc()
    in_maps = [
        {
            "segmentation": seg[c * B_PER : (c + 1) * B_PER],
            "shift_up": su,
            "shift_dn": sd,
        }
        for c in range(N_CORES)
    ]
    res = run_bass_kernel_spmd(nc, in_maps, core_ids=list(range(N_CORES)))
    xnms = np.concatenate([r["xnms"] for r in res.results], axis=0)
    return (xnms, segmentation)


# revision 10
# speedup vs baseline: 1.0928x; 1.0928x over previous
"""Trainium2 Bass kernel for nn_DetectionHead (class-margin NMS).

reference:
    x  = seg[:,1] - seg[:,0] - EPS
    xp = relu(x)
    xm = 3x3 hole max-pool of xp (8 neighbors, zero-padded), max'ed with 0
    out = x * (x > xm)                       ;  returns (out, seg)

Identity used: max(0, pool(relu(x))) == max(0, pool_{0-pad}(x)), so the pool
runs on raw x with zero padding and a single max(., 0) clamp — every value on
the comparison path stays bit-exact with the fp32 reference.

Sharding: pure data-parallel over batch (16 images -> 8 cores x 2). Per-core
layout: H on partitions in chunks of 126 output rows (+1 halo row each side),
both images side by side along the free dim (W columns each). Horizontal
neighbor maxes are shifted free-dim APs on a zero-padded x tile; vertical +-1
row shifts cross partitions, which compute engines cannot do, so they run on
the PE as fp32 matmuls with shifted-identity weights (permutation weights are
bit-exact through the 4-pass fp32 path).

Custom fused DVE ops (registered at import):
    MARGIN_SUB_NMS_ANT: x = (c1 - c0) - eps          (one op, ref rounding)
    MASK_MUL_NMS_ANT:   out = select(x > xm, x, 0)   (mask+mul in one op)
"""

import numpy as np

import concourse.bacc as bacc
import concourse.dve_ops as dve_ops
import concourse.mybir as mybir
from concourse.dve_ops import DveOp, get_dve_sub_opcode, has_src1
from concourse.dve_spec import Spec, Src0, Src1, C0, Zero, lower, select
from concourse.dve_uop import DveOpSpec
from concourse.tile import TileContext
from concourse.bass_utils import run_bass_kernel_spmd

EPS = 0.01
B, C, H, W = 16, 2, 1024, 1024
N_CORES = 8
B_PER = B // N_CORES  # images per core
CHUNK = 126  # output rows per full chunk (128 partitions - 2 halo rows)
WP = W + 2  # padded width

F32 = mybir.dt.float32
ALU = mybir.AluOpType


def _register_dve_op(name: str, spec: Spec) -> DveOp:
    for op in dve_ops.OPS:
        if op.name == name:
            return op
    op = DveOp(name, spec, subdim=False, uops_sha={})
    dve_ops.OPS.append(op)
    dve_ops._SUB_OPCODE_FOR_NAME[name] = dve_ops._CUSTOM_DVE_ROW_BASE + len(dve_ops.OPS) - 1
    dve_ops.CUSTOM_DVE_SPECS[name] = spec
    for ver in ("v3", "v4"):
        compiled = DveOpSpec(
            name=name,
            opcode=get_dve_sub_opcode(name),
            uops=lower(spec, ver=ver),
            rd1_en=has_src1(spec),
        )
        op.uops_sha[ver] = compiled.sha(ver)
    return op


MARGIN_SUB_NMS_ANT = _register_dve_op(
    "MARGIN_SUB_NMS_ANT",
    Spec(
        body=(Src0 - Src1) - C0,
        reference=lambda in0, in1, s0, s1, imm2: (in0 - in1) - s0,
    ),
)

MASK_MUL_NMS_ANT = _register_dve_op(
    "MASK_MUL_NMS_ANT",
    Spec(
        body=select(Src0 > Src1, Src0, Zero),
        reference=lambda in0, in1, s0, s1, imm2: np.where(in0 > in1, in0, np.float32(0.0)),
    ),
)


def _image_chunks():
    """Per-image chunk list: (r0, r1, or0, or1, po).

    rows [r0, r1) are loaded to partitions [0, r1-r0); output rows
    [or0, or1) live at partitions [po, po + (or1-or0)).
    """
    chunks = [(0, 127, 0, 126, 0)]
    for c in range(1, (H + CHUNK - 1) // CHUNK):
        s = CHUNK * c
        chunks.append((s - 1, min(s + CHUNK + 1, H), s, min(s + CHUNK, H), 1))
    return chunks


def _build_nc():
    nc = bacc.Bacc("TRN2", target_bir_lowering=False, debug=False)
    seg = nc.dram_tensor("segmentation", [B_PER, C, H, W], F32, kind="ExternalInput")
    su = nc.dram_tensor("shift_up", [128, 128], F32, kind="ExternalInput")
    sd = nc.dram_tensor("shift_dn", [128, 128], F32, kind="ExternalInput")
    out = nc.dram_tensor("xnms", [B_PER, H, W], F32, kind="ExternalOutput")

    seg_ap = seg.ap()
    out_ap = out.ap()

    with TileContext(nc) as tc:
        with (
            tc.tile_pool(name="wpool", bufs=1) as wpool,
            tc.tile_pool(name="inp", bufs=4) as inp,
            tc.tile_pool(name="xsp", bufs=3) as xsp,
            tc.tile_pool(name="tmp", bufs=3) as tmp,
            tc.tile_pool(name="psu", bufs=2, space="PSUM") as psu,
            tc.tile_pool(name="psd", bufs=2, space="PSUM") as psd,
            tc.tile_pool(name="outp", bufs=3) as outp,
        ):
            su_t = wpool.tile([128, 128], F32, tag="su")
            sd_t = wpool.tile([128, 128], F32, tag="sd")
            nc.sync.dma_start(out=su_t[:], in_=su.ap())
            nc.sync.dma_start(out=sd_t[:], in_=sd.ap())

            for b in range(B_PER):
                for (r0, r1, or0, or1, po) in _image_chunks():
                    nr = r1 - r0
                    orows = or1 - or0

                    # one DMA: both channels of one image -> [128, C*W]
                    it = inp.tile([128, C * W], F32, tag="it")
                    nc.sync.dma_start(
                        out=it[:nr, :].rearrange("p (c w) -> p c w", c=C),
                        in_=seg_ap[b].rearrange("c h w -> h c w")[r0:r1, :, :],
                    )

                    # x = (c1 - c0) - EPS, rounded exactly like the reference
                    xt = xsp.tile([128, W], F32, tag="xt")
                    nc.vector._custom_dve(
                        MARGIN_SUB_NMS_ANT,
                        out=xt[:nr, :],
                        in0=it[:nr, W : 2 * W],
                        in1=it[:nr, 0:W],
                        s0=EPS,
                    )

                    # t1 = max(x[j-1], x[j+1])  (0-padded at the image edges;
                    # the pad zero is absorbed by the global max(.,0) clamp)
                    t1 = tmp.tile([128, W], F32, tag="t1")
                    nc.vector.tensor_max(
                        t1[:nr, 1 : W - 1], xt[:nr, 0 : W - 2], xt[:nr, 2:W]
                    )
                    nc.vector.tensor_scalar_max(t1[:nr, 0:1], xt[:nr, 1:2], 0.0)
                    nc.vector.tensor_scalar_max(
                        t1[:nr, W - 1 : W], xt[:nr, W - 2 : W - 1], 0.0
                    )
                    # m3 = max(t1, x[j])
                    m3 = tmp.tile([128, W], F32, tag="m3")
                    nc.vector.tensor_max(m3[:nr], t1[:nr], xt[:nr])

                    # vertical +-1 shifts of m3 through the PE (exact fp32)
                    up = psu.tile([128, W], F32, tag="up")
                    dn = psd.tile([128, W], F32, tag="dn")
                    for h0 in range(0, W, 512):
                        nc.tensor.matmul(
                            up[:, h0 : h0 + 512],
                            su_t[:nr, :],
                            m3[:nr, h0 : h0 + 512],
                            start=True,
                            stop=True,
                        )
                    for h0 in range(0, W, 512):
                        nc.tensor.matmul(
                            dn[:, h0 : h0 + 512],
                            sd_t[:nr, :],
                            m3[:nr, h0 : h0 + 512],
                            start=True,
                            stop=True,
                        )

                    # xm = max(up, dn, t1, 0) — one PSUM operand per instruction
                    a = tmp.tile([128, W], F32, tag="a")
                    nc.vector.scalar_tensor_tensor(
                        a[:nr], up[:nr], 0.0, t1[:nr], ALU.max, ALU.max
                    )
                    xm = tmp.tile([128, W], F32, tag="xm")
                    nc.vector.tensor_max(xm[:nr], a[:nr], dn[:nr])

                    # out = x * (x > xm)
                    ot = outp.tile([128, W], F32, tag="ot")
                    nc.vector._custom_dve(
                        MASK_MUL_NMS_ANT,
                        out=ot[:nr, :],
                        in0=xt[:nr, :],
                        in1=xm[:nr, :],
                    )

                    # out-DMA on the ACT HWDGE ring (separate FIFO from input)
                    nc.scalar.dma_start(
                        out=out_ap[b, or0:or1, :],
                        in_=ot[po : po + orows, :],
                    )

    nc.compile()
    return nc


_NC = None


def _get_nc():
    global _NC
    if _NC is None:
        _NC = _build_nc()
    return _NC


def kernel(segmentation: np.ndarray) -> tuple[np.ndarray, np.ndarray]:
    seg = np.ascontiguousarray(segmentation, dtype=np.float32)
    su = np.eye(128, k=-1, dtype=np.float32)  # out[p] = in[p+1]
    sd = np.eye(128, k=1, dtype=np.float32)  # out[p] = in[p-1]
    nc = _get_nc()
    in_maps = [
        {
            "segmentation": seg[c * B_PER : (c + 1) * B_PER],
            "shift_up": su,
            "shift_dn": sd,
        }
        for c in range(N_CORES)
    ]
    res = run_bass_kernel_spmd(nc, in_maps, core_ids=list(range(N_CORES)))
    xnms = np.concatenate([r["xnms"] for r in res.results], axis=0)
    return (xnms, segmentation)


# revision 11
# speedup vs baseline: 1.2860x; 1.1768x over previous
"""Trainium2 Bass kernel for nn_DetectionHead (class-margin NMS).

reference:
    x  = seg[:,1] - seg[:,0] - EPS
    xp = relu(x)
    xm = 3x3 hole max-pool of xp (8 neighbors, zero-padded), max'ed with 0
    out = x * (x > xm)                       ;  returns (out, seg)

Identity used: max(0, pool(relu(x))) == max(0, pool_{0-pad}(x)), so the pool
runs on raw x with zero padding and a single max(., 0) clamp — every value on
the comparison path stays bit-exact with the fp32 reference.

Sharding: pure data-parallel over batch (16 images -> 8 cores x 2).

Per-core layout: TWO image rows per partition (partition p of a chunk holds
rows r0+2p and r0+2p+1, side by side along the free dim). This makes one of
the two vertical neighbors of every row a same-partition free-dim access;
only one cross-partition shift per row parity remains, so the PE shift work
(exact fp32 matmuls with shifted-identity weights — permutation weights are
bit-exact through the multi-pass fp32 path) is half of a row-per-partition
layout, and HBM rows pair into 8 KiB contiguous DMA descriptors. Horizontal
neighbor maxes are shifted free-dim APs.

Custom fused DVE ops (registered at import):
    MARGIN_SUB_NMS_ANT: x = (c1 - c0) - eps          (one op, ref rounding)
    MASK_MUL_NMS_ANT:   out = select(x > xm, x, 0)   (mask+mul in one op)
"""

import numpy as np

import concourse.bacc as bacc
import concourse.dve_ops as dve_ops
import concourse.mybir as mybir
from concourse.dve_ops import DveOp, get_dve_sub_opcode, has_src1
from concourse.dve_spec import Spec, Src0, Src1, C0, Zero, lower, select
from concourse.dve_uop import DveOpSpec
from concourse.tile import TileContext
from concourse.bass_utils import run_bass_kernel_spmd

EPS = 0.01
B, C, H, W = 16, 2, 1024, 1024
N_CORES = 8
B_PER = B // N_CORES  # images per core

F32 = mybir.dt.float32
ALU = mybir.AluOpType


def _register_dve_op(name: str, spec: Spec) -> DveOp:
    for op in dve_ops.OPS:
        if op.name == name:
            return op
    op = DveOp(name, spec, subdim=False, uops_sha={})
    dve_ops.OPS.append(op)
    dve_ops._SUB_OPCODE_FOR_NAME[name] = (
        dve_ops._CUSTOM_DVE_ROW_BASE + len(dve_ops.OPS) - 1
    )
    dve_ops.CUSTOM_DVE_SPECS[name] = spec
    for ver in ("v3", "v4"):
        compiled = DveOpSpec(
            name=name,
            opcode=get_dve_sub_opcode(name),
            uops=lower(spec, ver=ver),
            rd1_en=has_src1(spec),
        )
        op.uops_sha[ver] = compiled.sha(ver)
    return op


MARGIN_SUB_NMS_ANT = _register_dve_op(
    "MARGIN_SUB_NMS_ANT",
    Spec(
        body=(Src0 - Src1) - C0,
        reference=lambda in0, in1, s0, s1, imm2: (in0 - in1) - s0,
    ),
)

MASK_MUL_NMS_ANT = _register_dve_op(
    "MASK_MUL_NMS_ANT",
    Spec(
        body=select(Src0 > Src1, Src0, Zero),
        reference=lambda in0, in1, s0, s1, imm2: np.where(
            in0 > in1, in0, np.float32(0.0)
        ),
    ),
)

# Full chunks per image: (r0, pn). Loads rows [r0, r0+2*pn), partition p
# holding rows r0+2p (slot 0) and r0+2p+1 (slot 1).
_FULL_CHUNKS = [(0, 128), (254, 128), (508, 128), (762, 128)]
_TAIL_R0 = 1016  # 8 rows per image; both images' tails merge into one chunk


def _build_nc():
    nc = bacc.Bacc("TRN2", target_bir_lowering=False, debug=False)
    seg = nc.dram_tensor("segmentation", [B_PER, C, H, W], F32, kind="ExternalInput")
    su = nc.dram_tensor("shift_up", [128, 128], F32, kind="ExternalInput")
    sd = nc.dram_tensor("shift_dn", [128, 128], F32, kind="ExternalInput")
    sut = nc.dram_tensor("shift_up_tail", [128, 128], F32, kind="ExternalInput")
    out = nc.dram_tensor("xnms", [B_PER, H, W], F32, kind="ExternalOutput")

    seg_ap = seg.ap()
    out_ap = out.ap()
    # paired-row views: [512, 2W] with paired-row h holding rows 2h, 2h+1
    segp = [
        [seg_ap[b, c].rearrange("(h s) w -> h (s w)", s=2) for c in range(C)]
        for b in range(B_PER)
    ]
    outp_v = [out_ap[b].rearrange("(h s) w -> h (s w)", s=2) for b in range(B_PER)]
    W2 = 2 * W

    with TileContext(nc) as tc:
        with (
            tc.tile_pool(name="wpool", bufs=1) as wpool,
            tc.tile_pool(name="inp", bufs=3) as inp,
            tc.tile_pool(name="xsp", bufs=2) as xsp,
            tc.tile_pool(name="tmp", bufs=2) as tmp,
            tc.tile_pool(name="psu", bufs=2, space="PSUM") as psu,
            tc.tile_pool(name="psd", bufs=2, space="PSUM") as psd,
            tc.tile_pool(name="outpool", bufs=3) as outpool,
        ):
            su_t = wpool.tile([128, 128], F32, tag="su")
            sd_t = wpool.tile([128, 128], F32, tag="sd")
            sut_t = wpool.tile([128, 128], F32, tag="sut")
            nc.sync.dma_start(out=su_t[:], in_=su.ap())
            nc.scalar.dma_start(out=sd_t[:], in_=sd.ap())
            nc.sync.dma_start(out=sut_t[:], in_=sut.ap())

            def chunk_body(loads, pn, up_w, emits):
                """loads: [(dst_p0, image, hp0, ring, n_p)]; pn: active
                partitions; emits: [((p0, p1, f0, f1), dst_ap)] out-DMAs."""
                it = inp.tile([128, 2 * W2], F32, tag="it")
                for dst_p0, bb, hp0, prt, n_p in loads:
                    dma0 = nc.sync if prt == 0 else nc.scalar
                    dma1 = nc.scalar if prt == 0 else nc.sync
                    dma0.dma_start(
                        out=it[dst_p0 : dst_p0 + n_p, 0:W2],
                        in_=segp[bb][0][hp0 : hp0 + n_p],
                    )
                    dma1.dma_start(
                        out=it[dst_p0 : dst_p0 + n_p, W2 : 2 * W2],
                        in_=segp[bb][1][hp0 : hp0 + n_p],
                    )

                xt = xsp.tile([128, W2], F32, tag="xt")
                nc.vector._custom_dve(
                    MARGIN_SUB_NMS_ANT,
                    out=xt[:pn, :],
                    in0=it[:pn, W2 : 2 * W2],
                    in1=it[:pn, 0:W2],
                    s0=EPS,
                )

                x3 = xt[:pn, :].rearrange("p (s w) -> p s w", s=2)
                t1 = tmp.tile([128, W2], F32, tag="t1")
                t13 = t1[:pn, :].rearrange("p (s w) -> p s w", s=2)
                nc.vector.tensor_max(
                    t13[:, :, 1 : W - 1], x3[:, :, 0 : W - 2], x3[:, :, 2:W]
                )
                nc.vector.tensor_scalar_max(t13[:, :, 0:1], x3[:, :, 1:2], 0.0)
                nc.vector.tensor_scalar_max(
                    t13[:, :, W - 1 : W], x3[:, :, W - 2 : W - 1], 0.0
                )
                m3 = tmp.tile([128, W2], F32, tag="m3")
                nc.vector.tensor_max(m3[:pn], t1[:pn], xt[:pn])

                # cross-partition shifts via PE: up of slot0, dn of slot1
                up = psu.tile([128, W], F32, tag="up")
                dn = psd.tile([128, W], F32, tag="dn")
                for h0 in range(0, W, 512):
                    nc.tensor.matmul(
                        up[:, h0 : h0 + 512],
                        up_w[:pn, :],
                        m3[:pn, h0 : h0 + 512],
                        start=True,
                        stop=True,
                    )
                for h0 in range(0, W, 512):
                    nc.tensor.matmul(
                        dn[:, h0 : h0 + 512],
                        sd_t[:pn, :],
                        m3[:pn, W + h0 : W + h0 + 512],
                        start=True,
                        stop=True,
                    )

                # xm slot0 = max(dn, m3 slot1, t1 slot0, 0)
                # xm slot1 = max(up, m3 slot0, t1 slot1, 0)
                xm = xsp.tile([128, W2], F32, tag="xm")
                a0 = tmp.tile([128, W], F32, tag="a0")
                nc.vector.scalar_tensor_tensor(
                    a0[:pn], dn[:pn], 0.0, m3[:pn, W:W2], ALU.max, ALU.max
                )
                nc.vector.tensor_max(xm[:pn, 0:W], a0[:pn], t1[:pn, 0:W])
                a1 = tmp.tile([128, W], F32, tag="a1")
                nc.vector.scalar_tensor_tensor(
                    a1[:pn], up[:pn], 0.0, m3[:pn, 0:W], ALU.max, ALU.max
                )
                nc.vector.tensor_max(xm[:pn, W:W2], a1[:pn], t1[:pn, W:W2])

                ot = outpool.tile([128, W2], F32, tag="ot")
                nc.vector._custom_dve(
                    MASK_MUL_NMS_ANT,
                    out=ot[:pn, :],
                    in0=xt[:pn, :],
                    in1=xm[:pn, :],
                )

                for i, (src, dst) in enumerate(emits):
                    dma = nc.sync if i % 2 == 0 else nc.scalar
                    dma.dma_start(out=dst, in_=ot[src[0] : src[1], src[2] : src[3]])

            for b in range(B_PER):
                for ci, (r0, pn) in enumerate(_FULL_CHUNKS):
                    hp0 = r0 // 2
                    loads = [(0, b, hp0, b % 2, pn)]
                    emits = []
                    if ci == 0:
                        # out rows 0..253 (p0..126 full) + row 254 (p127 s0)
                        emits.append(((0, 127, 0, W2), outp_v[b][0:127]))
                        emits.append(((127, 128, 0, W), out_ap[b, 254:255, :]))
                    else:
                        # row r0+1 (p0 s1) + rows r0+2..r0+253 (p1..126 full)
                        # + row r0+254 (p127 s0)
                        emits.append(((0, 1, W, W2), out_ap[b, r0 + 1 : r0 + 2, :]))
                        emits.append(((1, 127, 0, W2), outp_v[b][hp0 + 1 : hp0 + 127]))
                        emits.append(
                            ((127, 128, 0, W), out_ap[b, r0 + 254 : r0 + 255, :])
                        )
                    chunk_body(loads, pn, su_t, emits)

            # merged tail: image 0 rows 1016.. at partitions 0..3, image 1 at 4..7
            hp0 = _TAIL_R0 // 2
            loads = [(0, 0, hp0, 0, 4), (4, 1, hp0, 1, 4)]
            emits = []
            for b, p0 in ((0, 0), (1, 4)):
                emits.append(((p0, p0 + 1, W, W2), out_ap[b, 1017:1018, :]))
                emits.append(((p0 + 1, p0 + 4, 0, W2), outp_v[b][hp0 + 1 : hp0 + 4]))
            chunk_body(loads, 8, sut_t, emits)

    nc.compile()
    return nc


_NC = None


def _get_nc():
    global _NC
    if _NC is None:
        _NC = _build_nc()
    return _NC


def _weights():
    su = np.eye(128, k=-1, dtype=np.float32)  # up[p] = in[p+1]
    sd = np.eye(128, k=1, dtype=np.float32)  # dn[p] = in[p-1]
    sut = su.copy()
    sut[:, 3] = 0.0  # image-0 tail bottom pad
    sut[:, 7] = 0.0  # image-1 tail bottom pad
    return su, sd, sut


def kernel(segmentation: np.ndarray) -> tuple[np.ndarray, np.ndarray]:
    seg = np.ascontiguousarray(segmentation, dtype=np.float32)
    su, sd, sut = _weights()
    nc = _get_nc()
    in_maps = [
        {
            "segmentation": seg[c * B_PER : (c + 1) * B_PER],
            "shift_up": su,
            "shift_dn": sd,
            "shift_up_tail": sut,
        }
        for c in range(N_CORES)
    ]
    res = run_bass_kernel_spmd(nc, in_maps, core_ids=list(range(N_CORES)))
    xnms = np.concatenate([r["xnms"] for r in res.results], axis=0)
    return (xnms, segmentation)


# revision 12
# speedup vs baseline: 1.3797x; 1.0728x over previous
"""Trainium2 Bass kernel for nn_DetectionHead (class-margin NMS).

reference:
    x  = seg[:,1] - seg[:,0] - EPS
    xp = relu(x)
    xm = 3x3 hole max-pool of xp (8 neighbors, zero-padded), max'ed with 0
    out = x * (x > xm)                       ;  returns (out, seg)

Identity used: max(0, pool(relu(x))) == max(0, pool_{0-pad}(x)), so the pool
runs on raw x with zero padding and a single max(., 0) clamp — every value on
the comparison path stays bit-exact with the fp32 reference.

Sharding: pure data-parallel over batch (16 images -> 8 cores x 2).

Per-core layout: TWO image rows per partition (partition p of a chunk holds
rows r0+2p and r0+2p+1, side by side along the free dim). This makes one of
the two vertical neighbors of every row a same-partition free-dim access;
only one cross-partition shift per row parity remains, so the PE shift work
(exact fp32 matmuls with shifted-identity weights — permutation weights are
bit-exact through the multi-pass fp32 path) is half of a row-per-partition
layout, and HBM rows pair into 8 KiB contiguous DMA descriptors. Horizontal
neighbor maxes are shifted free-dim APs.

Custom fused DVE ops (registered at import):
    MARGIN_SUB_NMS_ANT: x = (c1 - c0) - eps          (one op, ref rounding)
    MASK_MUL_NMS_ANT:   out = select(x > xm, x, 0)   (mask+mul in one op)
"""

import numpy as np

import concourse.bacc as bacc
import concourse.dve_ops as dve_ops
import concourse.mybir as mybir
from concourse.dve_ops import DveOp, get_dve_sub_opcode, has_src1
from concourse.dve_spec import Spec, Src0, Src1, C0, Zero, lower, select
from concourse.dve_uop import DveOpSpec
from concourse.tile import TileContext
from concourse.bass_utils import run_bass_kernel_spmd

EPS = 0.01
B, C, H, W = 16, 2, 1024, 1024
N_CORES = 8
B_PER = B // N_CORES  # images per core

F32 = mybir.dt.float32
ALU = mybir.AluOpType


def _register_dve_op(name: str, spec: Spec) -> DveOp:
    for op in dve_ops.OPS:
        if op.name == name:
            return op
    op = DveOp(name, spec, subdim=False, uops_sha={})
    dve_ops.OPS.append(op)
    dve_ops._SUB_OPCODE_FOR_NAME[name] = (
        dve_ops._CUSTOM_DVE_ROW_BASE + len(dve_ops.OPS) - 1
    )
    dve_ops.CUSTOM_DVE_SPECS[name] = spec
    for ver in ("v3", "v4"):
        compiled = DveOpSpec(
            name=name,
            opcode=get_dve_sub_opcode(name),
            uops=lower(spec, ver=ver),
            rd1_en=has_src1(spec),
        )
        op.uops_sha[ver] = compiled.sha(ver)
    return op


MARGIN_SUB_NMS_ANT = _register_dve_op(
    "MARGIN_SUB_NMS_ANT",
    Spec(
        body=(Src0 - Src1) - C0,
        reference=lambda in0, in1, s0, s1, imm2: (in0 - in1) - s0,
    ),
)

MASK_MUL_NMS_ANT = _register_dve_op(
    "MASK_MUL_NMS_ANT",
    Spec(
        body=select(Src0 > Src1, Src0, Zero),
        reference=lambda in0, in1, s0, s1, imm2: np.where(
            in0 > in1, in0, np.float32(0.0)
        ),
    ),
)

# Full chunks per image: (r0, pn). Loads rows [r0, r0+2*pn), partition p
# holding rows r0+2p (slot 0) and r0+2p+1 (slot 1).
_FULL_CHUNKS = [(0, 128), (254, 128), (508, 128), (762, 128)]
_TAIL_R0 = 1016  # 8 rows per image; both images' tails merge into one chunk


def _build_nc():
    nc = bacc.Bacc("TRN2", target_bir_lowering=False, debug=False)
    seg = nc.dram_tensor("segmentation", [B_PER, C, H, W], F32, kind="ExternalInput")
    su = nc.dram_tensor("shift_up", [128, 128], F32, kind="ExternalInput")
    sd = nc.dram_tensor("shift_dn", [128, 128], F32, kind="ExternalInput")
    sut = nc.dram_tensor("shift_up_tail", [128, 128], F32, kind="ExternalInput")
    out = nc.dram_tensor("xnms", [B_PER, H, W], F32, kind="ExternalOutput")

    seg_ap = seg.ap()
    out_ap = out.ap()
    # paired-row views: [512, 2W] with paired-row h holding rows 2h, 2h+1
    segp = [
        [seg_ap[b, c].rearrange("(h s) w -> h (s w)", s=2) for c in range(C)]
        for b in range(B_PER)
    ]
    outp_v = [out_ap[b].rearrange("(h s) w -> h (s w)", s=2) for b in range(B_PER)]
    W2 = 2 * W

    with TileContext(nc) as tc:
        with (
            tc.tile_pool(name="wpool", bufs=1) as wpool,
            tc.tile_pool(name="inp", bufs=4) as inp,
            tc.tile_pool(name="xsp", bufs=2) as xsp,
            tc.tile_pool(name="tmp", bufs=2) as tmp,
            tc.tile_pool(name="psu", bufs=2, space="PSUM") as psu,
            tc.tile_pool(name="psd", bufs=2, space="PSUM") as psd,
            tc.tile_pool(name="outpool", bufs=3) as outpool,
        ):
            su_t = wpool.tile([128, 128], F32, tag="su")
            sd_t = wpool.tile([128, 128], F32, tag="sd")
            sut_t = wpool.tile([128, 128], F32, tag="sut")
            nc.sync.dma_start(out=su_t[:], in_=su.ap())
            nc.sync.dma_start(out=sd_t[:], in_=sd.ap())
            nc.sync.dma_start(out=sut_t[:], in_=sut.ap())

            def chunk_body(loads, pn, up_w, emits):
                """loads: [(dst_p0, image, hp0, ring, n_p)]; pn: active
                partitions; emits: [((p0, p1, f0, f1), dst_ap)] out-DMAs."""
                it = inp.tile([128, 2 * W2], F32, tag="it")
                for dst_p0, bb, hp0, prt, n_p in loads:
                    # all inputs ride the sync HWDGE ring; outputs ride the
                    # scalar ring — mixing them causes head-of-line blocking
                    nc.sync.dma_start(
                        out=it[dst_p0 : dst_p0 + n_p, 0:W2],
                        in_=segp[bb][0][hp0 : hp0 + n_p],
                    )
                    nc.sync.dma_start(
                        out=it[dst_p0 : dst_p0 + n_p, W2 : 2 * W2],
                        in_=segp[bb][1][hp0 : hp0 + n_p],
                    )

                xt = xsp.tile([128, W2], F32, tag="xt")
                nc.vector._custom_dve(
                    MARGIN_SUB_NMS_ANT,
                    out=xt[:pn, :],
                    in0=it[:pn, W2 : 2 * W2],
                    in1=it[:pn, 0:W2],
                    s0=EPS,
                )

                x3 = xt[:pn, :].rearrange("p (s w) -> p s w", s=2)
                t1 = tmp.tile([128, W2], F32, tag="t1")
                t13 = t1[:pn, :].rearrange("p (s w) -> p s w", s=2)
                nc.vector.tensor_max(
                    t13[:, :, 1 : W - 1], x3[:, :, 0 : W - 2], x3[:, :, 2:W]
                )
                nc.vector.tensor_scalar_max(t13[:, :, 0:1], x3[:, :, 1:2], 0.0)
                nc.vector.tensor_scalar_max(
                    t13[:, :, W - 1 : W], x3[:, :, W - 2 : W - 1], 0.0
                )
                m3 = tmp.tile([128, W2], F32, tag="m3")
                nc.vector.tensor_max(m3[:pn], t1[:pn], xt[:pn])

                # cross-partition shifts via PE: up of slot0, dn of slot1
                up = psu.tile([128, W], F32, tag="up")
                dn = psd.tile([128, W], F32, tag="dn")
                for h0 in range(0, W, 512):
                    nc.tensor.matmul(
                        up[:, h0 : h0 + 512],
                        up_w[:pn, :],
                        m3[:pn, h0 : h0 + 512],
                        start=True,
                        stop=True,
                    )
                for h0 in range(0, W, 512):
                    nc.tensor.matmul(
                        dn[:, h0 : h0 + 512],
                        sd_t[:pn, :],
                        m3[:pn, W + h0 : W + h0 + 512],
                        start=True,
                        stop=True,
                    )

                # xm slot0 = max(dn, m3 slot1, t1 slot0, 0)
                # xm slot1 = max(up, m3 slot0, t1 slot1, 0)
                xm = xsp.tile([128, W2], F32, tag="xm")
                a0 = tmp.tile([128, W], F32, tag="a0")
                nc.vector.scalar_tensor_tensor(
                    a0[:pn], dn[:pn], 0.0, m3[:pn, W:W2], ALU.max, ALU.max
                )
                nc.vector.tensor_max(xm[:pn, 0:W], a0[:pn], t1[:pn, 0:W])
                a1 = tmp.tile([128, W], F32, tag="a1")
                nc.vector.scalar_tensor_tensor(
                    a1[:pn], up[:pn], 0.0, m3[:pn, 0:W], ALU.max, ALU.max
                )
                nc.vector.tensor_max(xm[:pn, W:W2], a1[:pn], t1[:pn, W:W2])

                ot = outpool.tile([128, W2], F32, tag="ot")
                nc.vector._custom_dve(
                    MASK_MUL_NMS_ANT,
                    out=ot[:pn, :],
                    in0=xt[:pn, :],
                    in1=xm[:pn, :],
                )

                for src, dst in emits:
                    nc.scalar.dma_start(
                        out=dst, in_=ot[src[0] : src[1], src[2] : src[3]]
                    )

            for b in range(B_PER):
                for ci, (r0, pn) in enumerate(_FULL_CHUNKS):
                    hp0 = r0 // 2
                    loads = [(0, b, hp0, b % 2, pn)]
                    emits = []
                    if ci == 0:
                        # out rows 0..253 (p0..126 full) + row 254 (p127 s0)
                        emits.append(((0, 127, 0, W2), outp_v[b][0:127]))
                        emits.append(((127, 128, 0, W), out_ap[b, 254:255, :]))
                    else:
                        # row r0+1 (p0 s1) + rows r0+2..r0+253 (p1..126 full)
                        # + row r0+254 (p127 s0)
                        emits.append(((0, 1, W, W2), out_ap[b, r0 + 1 : r0 + 2, :]))
                        emits.append(((1, 127, 0, W2), outp_v[b][hp0 + 1 : hp0 + 127]))
                        emits.append(
                            ((127, 128, 0, W), out_ap[b, r0 + 254 : r0 + 255, :])
                        )
                    chunk_body(loads, pn, su_t, emits)

            # merged tail: image 0 rows 1016.. at partitions 0..3, image 1 at 4..7
            hp0 = _TAIL_R0 // 2
            loads = [(0, 0, hp0, 0, 4), (4, 1, hp0, 1, 4)]
            emits = []
            for b, p0 in ((0, 0), (1, 4)):
                emits.append(((p0, p0 + 1, W, W2), out_ap[b, 1017:1018, :]))
                emits.append(((p0 + 1, p0 + 4, 0, W2), outp_v[b][hp0 + 1 : hp0 + 4]))
            chunk_body(loads, 8, sut_t, emits)

    nc.compile()
    return nc


_NC = None


def _get_nc():
    global _NC
    if _NC is None:
        _NC = _build_nc()
    return _NC


def _weights():
    su = np.eye(128, k=-1, dtype=np.float32)  # up[p] = in[p+1]
    sd = np.eye(128, k=1, dtype=np.float32)  # dn[p] = in[p-1]
    sut = su.copy()
    sut[:, 3] = 0.0  # image-0 tail bottom pad
    sut[:, 7] = 0.0  # image-1 tail bottom pad
    return su, sd, sut


def kernel(segmentation: np.ndarray) -> tuple[np.ndarray, np.ndarray]:
    seg = np.ascontiguousarray(segmentation, dtype=np.float32)
    su, sd, sut = _weights()
    nc = _get_nc()
    in_maps = [
        {
            "segmentation": seg[c * B_PER : (c + 1) * B_PER],
            "shift_up": su,
            "shift_dn": sd,
            "shift_up_tail": sut,
        }
        for c in range(N_CORES)
    ]
    res = run_bass_kernel_spmd(nc, in_maps, core_ids=list(range(N_CORES)))
    xnms = np.concatenate([r["xnms"] for r in res.results], axis=0)
    return (xnms, segmentation)
